# revision 1
# baseline (speedup 1.0000x reference)
"""GSAPool pairwise-distance + mean-threshold adjacency kernel for TRN2.

dist[b,i,j] = sqrt(||x_i||^2 + ||y_j||^2 - 2 x_i.y_j), mask = dist <= mean_b(dist)

Sharding: pure data-parallel over batch b: 64 samples -> 8 cores x 8 samples.
Per sample on a core:
  - load x,y [1024,256] natural layout
  - xx[m] row norms via DVE tensor_tensor_reduce (mult+add)
  - PE-transpose x,y to d-major; x side scaled by -2 on copy-out
  - yy[n] via ones-matmul over ysq (partition reduction on PE)
  - dist^2 psum = (-2x)^T y  (+ rank-1 ones x yy row), K=256 in 2 k-tiles
  - ACT: dist = sqrt(psum + xx bias), fused accum_out row sums for the mean
  - mean via ones-matmul + reduce + broadcast matmul
  - DVE tensor_scalar is_le -> u8 mask
Outputs: dist f32, mask u8 (cast to bool on host).
"""

import numpy as np
from contextlib import ExitStack

import concourse.bass as bass
import concourse.tile as tile
from concourse import bacc, mybir
from concourse.bass_utils import run_bass_kernel_spmd
from concourse.masks import make_identity

B = 64
M = 1024
N = 1024
D = 256
P = 128
MT = M // P        # 8 m-tiles
NCORES = 8
S = B // NCORES    # 8 samples per core
F32 = mybir.dt.float32
U8 = mybir.dt.uint8
ALU = mybir.AluOpType
ACTF = mybir.ActivationFunctionType


def build_body(ctx, tc, x_d, y_d, dist_d, mask_d, n_samples):
    nc = tc.nc

    const_pool = ctx.enter_context(tc.tile_pool(name="const", bufs=1))
    ident = const_pool.tile([P, P], F32)
    make_identity(nc, ident[:])
    ones_col = const_pool.tile([P, 8], F32)
    nc.gpsimd.memset(ones_col[:], 1.0)
    # [2, P] weights: row0 = ones, row1 = zeros — K=2 stand-in for rank-1
    # updates (K=1 matmuls are an unusual PE shape; avoid them).
    ones_row2 = const_pool.tile([2, P], F32)
    nc.gpsimd.memset(ones_row2[:, :], 0.0)
    nc.gpsimd.memset(ones_row2[0:1, :], 1.0)
    zeros_bias = const_pool.tile([P, 1], F32)
    nc.gpsimd.memset(zeros_bias[:], 0.0)

    nat_pool = ctx.enter_context(tc.tile_pool(name="nat", bufs=2))
    tr_pool = ctx.enter_context(tc.tile_pool(name="tr", bufs=2))
    dist_pool = ctx.enter_context(tc.tile_pool(name="dist", bufs=10))
    mask_pool = ctx.enter_context(tc.tile_pool(name="mask", bufs=2))
    small_pool = ctx.enter_context(tc.tile_pool(name="small", bufs=2))
    psum_tr = ctx.enter_context(tc.tile_pool(name="psum_tr", bufs=3, space="PSUM"))
    psum_d2 = ctx.enter_context(tc.tile_pool(name="psum_d2", bufs=3, space="PSUM"))
    psum_sm = ctx.enter_context(tc.tile_pool(name="psum_sm", bufs=2, space="PSUM"))

    for s in range(n_samples):
        # ---- loads (natural layout, m-tile t at free cols [t*D, (t+1)*D)) ----
        x_nat = nat_pool.tile([P, MT * D], F32, tag="x_nat")
        nc.sync.dma_start(
            out=x_nat.rearrange("p (t d) -> p t d", t=MT),
            in_=x_d[s].rearrange("(t p) d -> p t d", p=P),
        )
        y_nat = nat_pool.tile([P, MT * D], F32, tag="y_nat")
        nc.sync.dma_start(
            out=y_nat.rearrange("p (t d) -> p t d", t=MT),
            in_=y_d[s].rearrange("(t p) d -> p t d", p=P),
        )

        # ---- xx row norms: xx8[p, t] = sum_d x[128t+p, d]^2 ----
        # (ACT Square + fused accumulate; tensor_tensor_reduce faults the
        #  exec unit through this compile path, so keep it off.)
        xx8 = small_pool.tile([P, MT], F32, tag="xx8")
        for t in range(MT):
            sq_scratch = small_pool.tile([P, D], F32, tag="sq_scratch")
            nc.scalar.activation(
                sq_scratch[:],
                x_nat[:, t * D:(t + 1) * D],
                ACTF.Square,
                bias=zeros_bias[:, 0:1],
                scale=1.0,
                accum_out=xx8[:, t:t + 1],
            )

        # ---- PE transposes to d-major; x scaled by -2 on copy-out ----
        xTm2 = tr_pool.tile([P, 2 * M], F32, tag="xTm2")  # k-tile kt at cols [kt*M, (kt+1)*M)
        yT = tr_pool.tile([P, 2 * N], F32, tag="yT")
        ysq = tr_pool.tile([P, 2 * N], F32, tag="ysq")
        for kt in range(2):
            for t in range(MT):
                ptrx = psum_tr.tile([P, P], F32, tag="ptr")
                nc.tensor.transpose(
                    ptrx[:],
                    x_nat[:, t * D + kt * P: t * D + kt * P + P],
                    ident[:],
                )
                nc.vector.tensor_scalar_mul(
                    xTm2[:, kt * M + t * P: kt * M + (t + 1) * P], ptrx[:], -2.0
                )
        for kt in range(2):
            for t in range(MT):
                ptry = psum_tr.tile([P, P], F32, tag="ptr")
                nc.tensor.transpose(
                    ptry[:],
                    y_nat[:, t * D + kt * P: t * D + kt * P + P],
                    ident[:],
                )
                nc.vector.tensor_copy(
                    yT[:, kt * N + t * P: kt * N + (t + 1) * P], ptry[:]
                )
                nc.scalar.square(
                    ysq[:, kt * N + t * P: kt * N + (t + 1) * P], ptry[:]
                )

        # ---- yy row [2, N] via ones-matmul over ysq (row1 zeroed) ----
        yyrow = small_pool.tile([2, N], F32, tag="yyrow")
        nc.gpsimd.memset(yyrow[:, :], 0.0)
        for nh in range(2):
            pyy = psum_sm.tile([8, 512], F32, tag="sm")
            for kt in range(2):
                nc.tensor.matmul(
                    pyy[:],
                    ones_col[:],
                    ysq[:, kt * N + nh * 512: kt * N + nh * 512 + 512],
                    start=(kt == 0),
                    stop=(kt == 1),
                )
            nc.scalar.copy(yyrow[0:1, nh * 512:(nh + 1) * 512], pyy[0:1, :])

        # ---- main matmuls + fused sqrt/rowsum ----
        rs = small_pool.tile([P, 2 * MT], F32, tag="rs")
        dist_tiles = []
        for i in range(MT):
            dt_tile = dist_pool.tile([P, N], F32, tag="dist")
            for nh in range(2):
                pd2 = psum_d2.tile([P, 512], F32, tag="pd2")
                for kt in range(2):
                    nc.tensor.matmul(
                        pd2[:],
                        xTm2[:, kt * M + i * P: kt * M + (i + 1) * P],
                        yT[:, kt * N + nh * 512: kt * N + nh * 512 + 512],
                        start=(kt == 0),
                        stop=False,
                    )
                nc.tensor.matmul(
                    pd2[:],
                    ones_row2[:],
                    yyrow[:, nh * 512:(nh + 1) * 512],
                    start=False,
                    stop=True,
                )
                nc.scalar.activation(
                    dt_tile[:, nh * 512:(nh + 1) * 512],
                    pd2[:],
                    ACTF.Sqrt,
                    bias=xx8[:, i:i + 1],
                    scale=1.0,
                    accum_out=rs[:, 2 * i + nh: 2 * i + nh + 1],
                )
            nc.sync.dma_start(out=dist_d[s, i * P:(i + 1) * P, :], in_=dt_tile[:])
            dist_tiles.append(dt_tile)

        # ---- mean: total = sum(rs) over partitions and free ----
        ptot = psum_sm.tile([8, 2 * MT], F32, tag="sm")
        nc.tensor.matmul(ptot[:], ones_col[:], rs[:], start=True, stop=True)
        tot = small_pool.tile([2, 8], F32, tag="tot")
        nc.gpsimd.memset(tot[:, :], 0.0)
        nc.vector.tensor_reduce(
            out=tot[0:1, 0:1], in_=ptot[0:1, :], axis=mybir.AxisListType.X, op=ALU.add
        )
        pavg = psum_sm.tile([P, 8], F32, tag="sm")
        nc.tensor.matmul(pavg[:], ones_row2[:], tot[:], start=True, stop=True)
        avg = small_pool.tile([P, 1], F32, tag="avg")
        nc.scalar.activation(
            avg[:], pavg[:, 0:1], ACTF.Copy, bias=0.0, scale=1.0 / float(M * N)
        )

        # ---- compare + mask out ----
        mask_all = mask_pool.tile([P, MT * N], U8, tag="mask")
        for i in range(MT):
            nc.vector.tensor_scalar(
                mask_all[:, i * N:(i + 1) * N],
                dist_tiles[i][:],
                avg[:, 0:1],
                None,
                ALU.is_le,
            )
        nc.sync.dma_start(
            out=mask_d[s].rearrange("(t p) n -> p t n", p=P),
            in_=mask_all.rearrange("p (t n) -> p t n", t=MT),
        )


def build_program(n_samples=S, num_devices=NCORES):
    nc = bacc.Bacc(
        "TRN2", target_bir_lowering=False, debug=False, num_devices=num_devices
    )
    x_d = nc.dram_tensor("x", [n_samples, M, D], F32, kind="ExternalInput").ap()
    y_d = nc.dram_tensor("y", [n_samples, N, D], F32, kind="ExternalInput").ap()
    dist_d = nc.dram_tensor("dist", [n_samples, M, N], F32, kind="ExternalOutput").ap()
    mask_d = nc.dram_tensor("mask", [n_samples, M, N], U8, kind="ExternalOutput").ap()
    with tile.TileContext(nc) as tc:
        with ExitStack() as ctx:
            build_body(ctx, tc, x_d, y_d, dist_d, mask_d, n_samples)
    nc.compile()
    return nc


_nc_cache = None


def _get_nc():
    global _nc_cache
    if _nc_cache is None:
        _nc_cache = build_program()
    return _nc_cache


def kernel(x, y):
    x = np.ascontiguousarray(np.asarray(x), dtype=np.float32).reshape(B, M, D)
    y = np.ascontiguousarray(np.asarray(y), dtype=np.float32).reshape(B, N, D)
    nc = _get_nc()
    in_maps = [
        {
            "x": np.ascontiguousarray(x[c * S:(c + 1) * S]),
            "y": np.ascontiguousarray(y[c * S:(c + 1) * S]),
        }
        for c in range(NCORES)
    ]
    res = run_bass_kernel_spmd(nc, in_maps, list(range(NCORES)))
    dist = np.concatenate([res.results[c]["dist"] for c in range(NCORES)], axis=0)
    mask = np.concatenate([res.results[c]["mask"] for c in range(NCORES)], axis=0)
    return dist, mask != 0

